# revision 22
# baseline (speedup 1.0000x reference)
"""Causal self-attention (GQA + RoPE) Trainium2 Bass kernel.

Sharding: 8 cores = batch(2) x kv-group(4). Each core computes its batch's
4 q-heads / 1 kv-head and a row-shard of the Wo projection; the 4 partial
outputs per batch are summed on host (all-reduce replacement).

Self-contained: hardcodes all shapes from the problem spec.
"""

import numpy as np

import concourse.bass as bass
import concourse.mybir as mybir
from concourse.tile import TileContext
from concourse.bass_utils import run_bass_kernel_spmd

F32 = mybir.dt.float32
F32R = mybir.dt.float32r
BF16 = mybir.dt.bfloat16

B, T, C = 2, 2048, 1024
H, HKV, D = 16, 4, 64
HALF = D // 2  # 32
GQ = H // HKV  # 4 q heads per group
FQ = GQ * D    # 256 q features per group
FPROJ = FQ + 2 * D  # 384: q(256) + k(64) + v(64)
NT = T // 512  # 4 column blocks of 512
KT = C // 128  # 8 contraction tiles
MT = FPROJ // 128  # 3 output row tiles (q01, q23, kv)
NEG = -1.0e9


def _split_excess_waits(nc, max_waits=1):
    """walrus here encodes at most one sync-wait per instruction; hoist the
    rest into standalone EventSemaphore instructions (raw-bass encoding)."""
    n = 0
    for fn in nc.m.functions:
        for bb in fn.blocks:
            new = []
            changed = False
            for inst in bb.instructions:
                si = inst.sync_info
                if si is not None and len(si.on_wait) > max_waits:
                    waits = list(si.on_wait)
                    for j, w in enumerate(waits[max_waits:]):
                        ev = mybir.InstEventSemaphore(
                            name=f"{inst.name}-ws{j}",
                            engine=inst.engine,
                            ins=[],
                            outs=[],
                            sync_info=mybir.SyncInfo(on_wait=[w], on_update=[]),
                        )
                        new.append(ev)
                        n += 1
                    inst.sync_info = mybir.SyncInfo(
                        on_wait=waits[:max_waits], on_update=list(si.on_update)
                    )
                    changed = True
                new.append(inst)
            if changed:
                bb.instructions = new
    return n


def _build():
    nc = bass.Bass()
    xt_d = nc.dram_tensor("xt", [C, T], F32, kind="ExternalInput")
    wproj_d = nc.dram_tensor("wproj", [C, FPROJ], F32, kind="ExternalInput")
    wo_d = nc.dram_tensor("wo", [FQ, C], F32, kind="ExternalInput")
    atab_d = nc.dram_tensor("atab", [128, T], F32, kind="ExternalInput")
    btab_d = nc.dram_tensor("btab", [128, T], F32, kind="ExternalInput")
    pswap_d = nc.dram_tensor("pswap", [128, 128], F32, kind="ExternalInput")
    trib_d = nc.dram_tensor("trib", [128, 128], BF16, kind="ExternalInput")
    identb_d = nc.dram_tensor("identb", [128, 128], BF16, kind="ExternalInput")
    identr_d = nc.dram_tensor("identr", [64, 64], F32, kind="ExternalInput")
    ones65_d = nc.dram_tensor("ones65", [65, 64], F32, kind="ExternalInput")
    ones16_d = nc.dram_tensor("ones16", [128, 16], F32, kind="ExternalInput")
    zeros_d = nc.dram_tensor("zeros64", [64, T], F32, kind="ExternalInput")
    y_d = nc.dram_tensor("y", [T, C], F32, kind="ExternalOutput")

    with TileContext(nc) as tc:
        from contextlib import ExitStack

        with ExitStack() as ctx:
            const = ctx.enter_context(tc.tile_pool(name="const", bufs=1))
            pers = ctx.enter_context(tc.tile_pool(name="pers", bufs=1))
            # --- constants ---
            wproj_sb = const.tile([128, KT, FPROJ], F32R)
            wproj_r = wproj_d.rearrange("(ko p) f -> p ko f", p=128).bitcast(F32R)
            wo_sb = const.tile([128, 2, C], F32R)
            atab = const.tile([128, T], F32)
            btab = const.tile([128, T], F32)
            pswap = const.tile([128, 128], F32R)
            trib = const.tile([128, 128], BF16)
            identb = const.tile([128, 128], BF16)
            identr = const.tile([128, 64], F32R)
            ones65 = const.tile([65, 64], F32R)

            # --- persistent activations ---
            qr = [pers.tile([128, T], F32R, name=f"qr{i}") for i in range(2)]
            # k^T zero-padded to 128 contraction rows: kr0 = [k; 0] for even
            # heads, kr1 = [0; k] for odd heads -> S matmuls engage the full
            # PE array (HAM stays warm) while the zero half kills the other
            # head's q rows exactly.
            kr0 = pers.tile([128, T], F32R)
            kr1 = pers.tile([128, T], F32R)
            vsb = pers.tile([128, T // 128, 65], F32R)  # v natural + ones col
            opk = pers.tile([128, 2, T], F32R)  # packed normalized O^T for Wo

            misc_ps = ctx.enter_context(
                tc.tile_pool(name="miscps", bufs=2, space="PSUM")
            )

            # ---------------- phase A+B: projections + RoPE ----------------
            with tc.tile_pool(name="xt", bufs=32) as xpool, tc.tile_pool(
                name="tmp", bufs=3
            ) as tmp, tc.tile_pool(name="paps", bufs=3, space="PSUM") as pa_ps:
                # plain copy of kv proj (v^T in rows 64:); dead after phase B
                kvp = tmp.tile([128, T], F32R, tag="kvp", name="kvp")
                xts = {}
                for n in range(NT):
                    for k in range(KT):
                        if n == 0:
                            nc.sync.dma_start(wproj_sb[:, k], wproj_r[:, k])
                        t = xpool.tile([128, 512], F32R, tag="x", name=f"x{k}_{n}")
                        nc.sync.dma_start(
                            t[:], xt_d[bass.ts(k, 128), bass.ts(n, 512)].bitcast(F32R)
                        )
                        xts[(k, n)] = t
                    if n == 0:
                        # small constants (cheap), then RoPE tables
                        nc.sync.dma_start(pswap[:], pswap_d[:].bitcast(F32R))
                        nc.sync.dma_start(identr[64:128, :], identr_d[:].bitcast(F32R))
                        nc.sync.dma_start(trib[:], trib_d[:])
                        nc.sync.dma_start(identb[:], identb_d[:])
                        nc.sync.dma_start(ones65[:], ones65_d[:].bitcast(F32R))
                        nc.sync.dma_start(atab[:], atab_d[:])
                        nc.sync.dma_start(btab[:], btab_d[:])
                    if n == 1:
                        nc.sync.dma_start(kr0[64:128, :], zeros_d[:].bitcast(F32R))
                        nc.sync.dma_start(kr1[0:64, :], zeros_d[:].bitcast(F32R))
                        nc.sync.dma_start(vsb[:, :, 64], ones16_d[:].bitcast(F32R))
                        nc.sync.dma_start(
                            wo_sb[:],
                            wo_d.rearrange("(ko p) c -> p ko c", p=128).bitcast(F32R),
                        )

                for n in range(NT):
                    for m in range(MT):
                        ps = pa_ps.tile([128, 512], F32, tag="proj")
                        for k in range(KT):
                            nc.tensor.matmul(
                                ps[:],
                                wproj_sb[:, k, bass.ts(m, 128)],
                                xts[(k, n)][:],
                                start=(k == 0),
                                stop=(k == KT - 1),
                            )
                        # RoPE on this [128, 512] block
                        rows = 128 if m < 2 else 64
                        if m == 2:
                            plain = kvp[:, bass.ts(n, 512)]
                        else:
                            qt_t = tmp.tile([128, 512], F32R, tag="qtmp", name="qtmp")
                            plain = qt_t[:]
                        nc.vector.tensor_copy(plain, ps[:])
                        qsw = pa_ps.tile([128, 512], F32, tag="qsw")
                        nc.tensor.matmul(
                            qsw[0:rows],
                            pswap[0:rows, 0:rows],
                            plain[0:rows],
                            start=True,
                            stop=True,
                        )
                        t1 = tmp.tile([128, 512], F32, tag="t1")
                        nc.vector.tensor_tensor(
                            t1[0:rows],
                            ps[0:rows],
                            atab[0:rows, bass.ts(n, 512)],
                            mybir.AluOpType.mult,
                        )
                        t2 = tmp.tile([128, 512], F32, tag="t2")
                        nc.vector.tensor_tensor(
                            t2[0:rows],
                            qsw[0:rows],
                            btab[0:rows, bass.ts(n, 512)],
                            mybir.AluOpType.mult,
                        )
                        dest = qr[m] if m < 2 else kr0
                        nc.gpsimd.tensor_tensor(
                            dest[0:rows, bass.ts(n, 512)],
                            t1[0:rows],
                            t2[0:rows],
                            mybir.AluOpType.add,
                        )
                        if m == 2:
                            # duplicate k^T into kr1 rows 64:128
                            nc.vector.tensor_copy(
                                kr1[64:128, bass.ts(n, 512)],
                                kr0[0:64, bass.ts(n, 512)],
                            )
                            # v^T -> v natural (PE transpose per 128-token block)
                            for tt in range(4 * n, 4 * n + 4):
                                vt_ps = misc_ps.tile([128, 64], F32, tag="misc")
                                nc.tensor.transpose(
                                    vt_ps[:],
                                    kvp[64:128, bass.ts(tt, 128)].bitcast(F32),
                                    identr[64:128, :].bitcast(F32),
                                )
                                nc.vector.tensor_copy(vsb[:, tt, 0:64], vt_ps[:])

            # ---------------- phase C: attention ----------------
            spool = ctx.enter_context(tc.tile_pool(name="sps", bufs=2, space="PSUM"))
            opool = ctx.enter_context(tc.tile_pool(name="ops", bufs=2, space="PSUM"))
            ppool = ctx.enter_context(tc.tile_pool(name="pp", bufs=4))
            o65pool = ctx.enter_context(tc.tile_pool(name="o65p", bufs=8))
            rcpool = ctx.enter_context(tc.tile_pool(name="rc", bufs=2))
            ypool = ctx.enter_context(tc.tile_pool(name="yp", bufs=4))
            o65_all = {}

            for hp in range(2):
                qtile = qr[hp]
                heads = (2 * hp, 2 * hp + 1)
                for jq in range(NT):
                    o_ps = {
                        h: opool.tile([65, 512], F32, tag="o", name=f"o{h}")
                        for h in heads
                    }
                    nkb = 4 * (jq + 1)
                    for ksb in range(nkb // 2):
                        regions = []
                        for jk in range(2):
                            kb = 2 * ksb + jk
                            j = kb - 4 * jq
                            col0 = max(j, 0) * 128
                            regions.append((jk, col0, kb))
                        s_ps = {}
                        p_sb = {}
                        for h in heads:
                            s_ps[h] = spool.tile([128, 1024], F32, tag="s", name=f"s{h}")
                            p_sb[h] = ppool.tile([128, 1024], F32R, tag="p", name=f"pb{h}")
                        for jk, col0, kb in regions:
                            for h in heads:
                                krt = kr0 if h % 2 == 0 else kr1
                                nc.tensor.matmul(
                                    s_ps[h][:, jk * 512 + col0 : jk * 512 + 512],
                                    krt[:, bass.ts(kb, 128)],
                                    qtile[:, jq * 512 + col0 : jq * 512 + 512],
                                    start=True,
                                    stop=(kb < 4 * jq),
                                )
                        for h in heads:
                            for jk, col0, kb in regions:
                                if kb - 4 * jq >= 0:
                                    nc.tensor.matmul(
                                        s_ps[h][:, jk * 512 + col0 : jk * 512 + col0 + 128],
                                        identb[:],
                                        trib[:],
                                        start=False,
                                        stop=True,
                                    )
                        for h in heads:
                            if regions[0][1] == 0 and regions[1][1] == 0:
                                nc.scalar.activation(
                                    p_sb[h][:],
                                    s_ps[h][:],
                                    mybir.ActivationFunctionType.Exp,
                                    scale=0.125,
                                )
                            else:
                                for jk, col0, kb in regions:
                                    nc.scalar.activation(
                                        p_sb[h][:, jk * 512 + col0 : jk * 512 + 512],
                                        s_ps[h][:, jk * 512 + col0 : jk * 512 + 512],
                                        mybir.ActivationFunctionType.Exp,
                                        scale=0.125,
                                    )
                        for h in heads:
                            for jk, col0, kb in regions:
                                nc.tensor.matmul(
                                    o_ps[h][:, col0:512],
                                    vsb[:, kb, :],
                                    p_sb[h][:, jk * 512 + col0 : jk * 512 + 512],
                                    start=(kb == 0),
                                    stop=(kb == nkb - 1),
                                )
                    for h in heads:
                        o65t = o65pool.tile(
                            [65, 512], F32R, tag="o65", name=f"o65_{h}_{jq}"
                        )
                        nc.vector.tensor_copy(o65t[:], o_ps[h][:])
                        o65_all[(h, jq)] = o65t

                # normalization for this head pair (overlaps the next pair's
                # attention)
                for h in heads:
                    for jq in range(NT):
                        o65t = o65_all[(h, jq)]
                        lnd = rcpool.tile([65, 512], F32, tag="lnd", name=f"ln{h}{jq}")
                        nc.scalar.activation(
                            lnd[64:65, :],
                            o65t[64:65, :].bitcast(F32),
                            mybir.ActivationFunctionType.Ln,
                        )
                        rc = rcpool.tile([65, 512], F32R, tag="rc", name=f"rc{h}{jq}")
                        nc.scalar.activation(
                            rc[64:65, :],
                            lnd[64:65, :],
                            mybir.ActivationFunctionType.Exp,
                            scale=-1.0,
                        )
                        bc_ps = misc_ps.tile([64, 512], F32, tag="misc", name=f"bc{h}{jq}")
                        nc.tensor.matmul(
                            bc_ps[:],
                            ones65[64:65, :],
                            rc[64:65, :],
                            start=True,
                            stop=True,
                            tile_position=(64, 0),
                        )
                        nc.vector.tensor_tensor(
                            opk[(h % 2) * 64 : (h % 2) * 64 + 64, h // 2, bass.ts(jq, 512)],
                            o65t[0:64, :].bitcast(F32),
                            bc_ps[:],
                            mybir.AluOpType.mult,
                        )
                # Wo contribution of this head pair (k-tile = hp), accumulated
                # into the zero-initialized DRAM output by SWDGE accum-DMA
                for t in range(T // 128):
                    for nn in range(2):
                        wps = misc_ps.tile([128, 512], F32, tag="misc", name="wps")
                        nc.tensor.matmul(
                            wps[:],
                            opk[:, hp, bass.ts(t, 128)],
                            wo_sb[:, hp, bass.ts(nn, 512)],
                            start=True,
                            stop=True,
                        )
                        y_sb = ypool.tile([128, 512], F32, tag="y", name="ysb")
                        if (t + nn) % 2 == 0:
                            nc.vector.tensor_copy(y_sb[:], wps[:])
                        else:
                            nc.scalar.copy(y_sb[:], wps[:])
                        nc.gpsimd.dma_start(
                            y_d[bass.ts(t, 128), bass.ts(nn, 512)],
                            y_sb[:],
                            accum_op=mybir.AluOpType.add,
                        )

            # (phase C2 + D are emitted per head-pair inside the hp loop)

    _split_excess_waits(nc)
    return nc


_NC_CACHE = None


def _get_nc():
    global _NC_CACHE
    if _NC_CACHE is None:
        _NC_CACHE = _build()
    return _NC_CACHE


def _host_prep(x, cos, sin, Wq, Wk, Wv, Wo):
    cos2 = np.asarray(cos, np.float32).reshape(T, HALF)  # [T, 32]
    sin2 = np.asarray(sin, np.float32).reshape(T, HALF)
    atab = np.tile(cos2.T, (4, 1))  # [128, T]
    btab = np.tile(np.vstack([sin2.T, -sin2.T]), (2, 1))  # [128, T]
    idx = np.arange(128)
    pswap = np.zeros((128, 128), np.float32)
    pswap[idx ^ 32, idx] = 1.0
    k_i = np.arange(128)[:, None]
    q_i = np.arange(128)[None, :]
    trib = np.where(k_i > q_i, np.float32(NEG), np.float32(0.0))
    import ml_dtypes

    trib = trib.astype(ml_dtypes.bfloat16)
    identb = np.eye(128, dtype=ml_dtypes.bfloat16)
    identr = np.eye(64, dtype=np.float32)
    ones65 = np.ones((65, 64), np.float32)
    ones16 = np.ones((128, 16), np.float32)
    zeros64 = np.zeros((64, T), np.float32)

    in_maps = []
    for core in range(8):
        b, g = core // 4, core % 4
        xt = np.ascontiguousarray(np.asarray(x[b], np.float32).T)  # [C, T]
        wproj = np.ascontiguousarray(
            np.concatenate(
                [
                    Wq[:, g * FQ : (g + 1) * FQ],
                    Wk[:, g * D : (g + 1) * D],
                    Wv[:, g * D : (g + 1) * D],
                ],
                axis=1,
            ).astype(np.float32)
        )
        wo = np.ascontiguousarray(Wo[g * FQ : (g + 1) * FQ, :].astype(np.float32))
        in_maps.append(
            {
                "xt": xt,
                "wproj": wproj,
                "wo": wo,
                "atab": atab,
                "btab": btab,
                "pswap": pswap,
                "trib": trib,
                "identb": identb,
                "identr": identr,
                "ones65": ones65,
                "ones16": ones16,
                "zeros64": zeros64,
            }
        )
    return in_maps


def kernel(x, cos, sin, Wq, Wk, Wv, Wo, _want_trace=False, _trace_kwargs=None):
    nc = _get_nc()
    in_maps = _host_prep(x, cos, sin, Wq, Wk, Wv, Wo)
    kw = {}
    if _want_trace:
        kw = dict(trace=True, **(_trace_kwargs or {}))
    res = run_bass_kernel_spmd(nc, in_maps, list(range(8)), **kw)
    y = np.zeros((B, T, C), np.float32)
    for core in range(8):
        b = core // 4
        y[b] += res.results[core]["y"]
    if _want_trace:
        kernel.last_result = res
    return y


# revision 24
# speedup vs baseline: 1.2220x; 1.2220x over previous
"""Causal self-attention (GQA + RoPE) Trainium2 Bass kernel.

Sharding: 8 cores = batch(2) x kv-group(4). Each core computes its batch's
4 q-heads / 1 kv-head and a row-shard of the Wo projection; the 4 partial
outputs per batch are summed on host (all-reduce replacement).

Self-contained: hardcodes all shapes from the problem spec.
"""

import numpy as np

import concourse.bass as bass
import concourse.mybir as mybir
from concourse.tile import TileContext
from concourse.bass_utils import run_bass_kernel_spmd

F32 = mybir.dt.float32
F32R = mybir.dt.float32r
BF16 = mybir.dt.bfloat16

B, T, C = 2, 2048, 1024
H, HKV, D = 16, 4, 64
HALF = D // 2  # 32
GQ = H // HKV  # 4 q heads per group
FQ = GQ * D    # 256 q features per group
FPROJ = FQ + 2 * D  # 384: q(256) + k(64) + v(64)
NT = T // 512  # 4 column blocks of 512
KT = C // 128  # 8 contraction tiles
MT = FPROJ // 128  # 3 output row tiles (q01, q23, kv)
NEG = -1.0e9


def _split_excess_waits(nc, max_waits=1):
    """walrus here encodes at most one sync-wait per instruction; hoist the
    rest into standalone EventSemaphore instructions (raw-bass encoding)."""
    n = 0
    for fn in nc.m.functions:
        for bb in fn.blocks:
            new = []
            changed = False
            for inst in bb.instructions:
                si = inst.sync_info
                if si is not None and len(si.on_wait) > max_waits:
                    waits = list(si.on_wait)
                    for j, w in enumerate(waits[max_waits:]):
                        ev = mybir.InstEventSemaphore(
                            name=f"{inst.name}-ws{j}",
                            engine=inst.engine,
                            ins=[],
                            outs=[],
                            sync_info=mybir.SyncInfo(on_wait=[w], on_update=[]),
                        )
                        new.append(ev)
                        n += 1
                    inst.sync_info = mybir.SyncInfo(
                        on_wait=waits[:max_waits], on_update=list(si.on_update)
                    )
                    changed = True
                new.append(inst)
            if changed:
                bb.instructions = new
    return n


def _build():
    nc = bass.Bass()
    xt_d = nc.dram_tensor("xt", [C, T], F32, kind="ExternalInput")
    wproj_d = nc.dram_tensor("wproj", [C, FPROJ], F32, kind="ExternalInput")
    wo_d = nc.dram_tensor("wo", [FQ, C], F32, kind="ExternalInput")
    atab_d = nc.dram_tensor("atab", [128, T], F32, kind="ExternalInput")
    btab_d = nc.dram_tensor("btab", [128, T], F32, kind="ExternalInput")
    pswap_d = nc.dram_tensor("pswap", [128, 128], F32, kind="ExternalInput")
    trib_d = nc.dram_tensor("trib", [128, 128], BF16, kind="ExternalInput")
    identb_d = nc.dram_tensor("identb", [128, 128], BF16, kind="ExternalInput")
    identr_d = nc.dram_tensor("identr", [64, 64], F32, kind="ExternalInput")
    ones65_d = nc.dram_tensor("ones65", [65, 64], F32, kind="ExternalInput")
    ones16_d = nc.dram_tensor("ones16", [128, 16], F32, kind="ExternalInput")
    zeros_d = nc.dram_tensor("zeros64", [64, T], F32, kind="ExternalInput")
    y_d = nc.dram_tensor("y", [T, C], F32, kind="ExternalOutput")

    with TileContext(nc) as tc:
        from contextlib import ExitStack

        with ExitStack() as ctx:
            const = ctx.enter_context(tc.tile_pool(name="const", bufs=1))
            pers = ctx.enter_context(tc.tile_pool(name="pers", bufs=1))
            # --- constants ---
            wproj_sb = const.tile([128, KT, FPROJ], F32R)
            wproj_r = wproj_d.rearrange("(ko p) f -> p ko f", p=128).bitcast(F32R)
            wo_sb = const.tile([128, 2, C], F32R)
            atab = const.tile([128, T], F32)
            btab = const.tile([128, T], F32)
            pswap = const.tile([128, 128], F32R)
            trib = const.tile([128, 128], BF16)
            identb = const.tile([128, 128], BF16)
            identr = const.tile([128, 64], F32R)
            ones65 = const.tile([65, 64], F32R)

            # --- persistent activations ---
            qr = [pers.tile([128, T], F32R, name=f"qr{i}") for i in range(2)]
            # k^T zero-padded to 128 contraction rows: kr0 = [k; 0] for even
            # heads, kr1 = [0; k] for odd heads -> S matmuls engage the full
            # PE array (HAM stays warm) while the zero half kills the other
            # head's q rows exactly.
            kr0 = pers.tile([128, T], F32R)
            kr1 = pers.tile([128, T], F32R)
            vsb = pers.tile([128, T // 128, 65], F32R)  # v natural + ones col
            opk = pers.tile([128, 2, T], F32R)  # packed normalized O^T for Wo

            misc_ps = ctx.enter_context(
                tc.tile_pool(name="miscps", bufs=2, space="PSUM")
            )

            # ---------------- phase A+B: projections + RoPE ----------------
            with tc.tile_pool(name="xt", bufs=32) as xpool, tc.tile_pool(
                name="tmp", bufs=3
            ) as tmp, tc.tile_pool(name="paps", bufs=3, space="PSUM") as pa_ps:
                # plain copy of kv proj (v^T in rows 64:); dead after phase B
                kvp = tmp.tile([128, T], F32R, tag="kvp", name="kvp")
                xts = {}
                for n in range(NT):
                    for k in range(KT):
                        if n == 0:
                            nc.sync.dma_start(wproj_sb[:, k], wproj_r[:, k])
                        t = xpool.tile([128, 512], F32R, tag="x", name=f"x{k}_{n}")
                        nc.sync.dma_start(
                            t[:], xt_d[bass.ts(k, 128), bass.ts(n, 512)].bitcast(F32R)
                        )
                        xts[(k, n)] = t
                    if n == 0:
                        # small constants (cheap), then RoPE tables
                        nc.sync.dma_start(pswap[:], pswap_d[:].bitcast(F32R))
                        nc.sync.dma_start(identr[64:128, :], identr_d[:].bitcast(F32R))
                        nc.sync.dma_start(trib[:], trib_d[:])
                        nc.sync.dma_start(identb[:], identb_d[:])
                        nc.sync.dma_start(ones65[:], ones65_d[:].bitcast(F32R))
                        nc.sync.dma_start(atab[:], atab_d[:])
                        nc.sync.dma_start(btab[:], btab_d[:])
                    if n == 1:
                        nc.sync.dma_start(kr0[64:128, :], zeros_d[:].bitcast(F32R))
                        nc.sync.dma_start(kr1[0:64, :], zeros_d[:].bitcast(F32R))
                        nc.sync.dma_start(vsb[:, :, 64], ones16_d[:].bitcast(F32R))
                        nc.sync.dma_start(
                            wo_sb[:],
                            wo_d.rearrange("(ko p) c -> p ko c", p=128).bitcast(F32R),
                        )

                for n in range(NT):
                    for m in range(MT):
                        ps = pa_ps.tile([128, 512], F32, tag="proj")
                        for k in range(KT):
                            nc.tensor.matmul(
                                ps[:],
                                wproj_sb[:, k, bass.ts(m, 128)],
                                xts[(k, n)][:],
                                start=(k == 0),
                                stop=(k == KT - 1),
                            )
                        # RoPE on this [128, 512] block
                        rows = 128 if m < 2 else 64
                        if m == 2:
                            plain = kvp[:, bass.ts(n, 512)]
                        else:
                            qt_t = tmp.tile([128, 512], F32R, tag="qtmp", name="qtmp")
                            plain = qt_t[:]
                        nc.vector.tensor_copy(plain, ps[:])
                        qsw = pa_ps.tile([128, 512], F32, tag="qsw")
                        nc.tensor.matmul(
                            qsw[0:rows],
                            pswap[0:rows, 0:rows],
                            plain[0:rows],
                            start=True,
                            stop=True,
                        )
                        t1 = tmp.tile([128, 512], F32, tag="t1")
                        nc.vector.tensor_tensor(
                            t1[0:rows],
                            ps[0:rows],
                            atab[0:rows, bass.ts(n, 512)],
                            mybir.AluOpType.mult,
                        )
                        t2 = tmp.tile([128, 512], F32, tag="t2")
                        nc.vector.tensor_tensor(
                            t2[0:rows],
                            qsw[0:rows],
                            btab[0:rows, bass.ts(n, 512)],
                            mybir.AluOpType.mult,
                        )
                        dest = qr[m] if m < 2 else kr0
                        nc.gpsimd.tensor_tensor(
                            dest[0:rows, bass.ts(n, 512)],
                            t1[0:rows],
                            t2[0:rows],
                            mybir.AluOpType.add,
                        )
                        if m == 2:
                            # duplicate k^T into kr1 rows 64:128
                            nc.vector.tensor_copy(
                                kr1[64:128, bass.ts(n, 512)],
                                kr0[0:64, bass.ts(n, 512)],
                            )
                            # v^T -> v natural (PE transpose per 128-token block)
                            for tt in range(4 * n, 4 * n + 4):
                                vt_ps = misc_ps.tile([128, 64], F32, tag="misc")
                                nc.tensor.transpose(
                                    vt_ps[:],
                                    kvp[64:128, bass.ts(tt, 128)].bitcast(F32),
                                    identr[64:128, :].bitcast(F32),
                                )
                                nc.vector.tensor_copy(vsb[:, tt, 0:64], vt_ps[:])

            # ---------------- phase C: attention ----------------
            attn_ctx = ExitStack()
            spool = attn_ctx.enter_context(
                tc.tile_pool(name="attnps", bufs=2, space="PSUM")
            )
            opool = spool  # same pool, separate tags
            ppool = ctx.enter_context(tc.tile_pool(name="pp", bufs=4))
            o65pool = ctx.enter_context(tc.tile_pool(name="o65p", bufs=8))
            rcpool = ctx.enter_context(tc.tile_pool(name="rc", bufs=2))
            ypool = ctx.enter_context(tc.tile_pool(name="yp", bufs=4))
            o65_all = {}

            for hp in range(2):
                qtile = qr[hp]
                heads = (2 * hp, 2 * hp + 1)
                for jq in range(NT):
                    o_ps = {
                        h: opool.tile([65, 512], F32, tag="o", name=f"o{h}")
                        for h in heads
                    }
                    nkb = 4 * (jq + 1)
                    for ksb in range(nkb // 2):
                        regions = []
                        for jk in range(2):
                            kb = 2 * ksb + jk
                            j = kb - 4 * jq
                            col0 = max(j, 0) * 128
                            regions.append((jk, col0, kb))
                        s_ps = {}
                        p_sb = {}
                        for h in heads:
                            s_ps[h] = spool.tile([128, 1024], F32, tag="s", name=f"s{h}")
                            p_sb[h] = ppool.tile([128, 1024], F32R, tag="p", name=f"pb{h}")
                        for jk, col0, kb in regions:
                            for h in heads:
                                krt = kr0 if h % 2 == 0 else kr1
                                nc.tensor.matmul(
                                    s_ps[h][:, jk * 512 + col0 : jk * 512 + 512],
                                    krt[:, bass.ts(kb, 128)],
                                    qtile[:, jq * 512 + col0 : jq * 512 + 512],
                                    start=True,
                                    stop=(kb < 4 * jq),
                                )
                        for h in heads:
                            for jk, col0, kb in regions:
                                if kb - 4 * jq >= 0:
                                    nc.tensor.matmul(
                                        s_ps[h][:, jk * 512 + col0 : jk * 512 + col0 + 128],
                                        identb[:],
                                        trib[:],
                                        start=False,
                                        stop=True,
                                    )
                        for h in heads:
                            if regions[0][1] == 0 and regions[1][1] == 0:
                                nc.scalar.activation(
                                    p_sb[h][:],
                                    s_ps[h][:],
                                    mybir.ActivationFunctionType.Exp,
                                    scale=0.125,
                                )
                            else:
                                for jk, col0, kb in regions:
                                    nc.scalar.activation(
                                        p_sb[h][:, jk * 512 + col0 : jk * 512 + 512],
                                        s_ps[h][:, jk * 512 + col0 : jk * 512 + 512],
                                        mybir.ActivationFunctionType.Exp,
                                        scale=0.125,
                                    )
                        for h in heads:
                            for jk, col0, kb in regions:
                                nc.tensor.matmul(
                                    o_ps[h][:, col0:512],
                                    vsb[:, kb, :],
                                    p_sb[h][:, jk * 512 + col0 : jk * 512 + 512],
                                    start=(kb == 0),
                                    stop=(kb == nkb - 1),
                                )
                    for h in heads:
                        o65t = o65pool.tile(
                            [65, 512], F32R, tag="o65", name=f"o65_{h}_{jq}"
                        )
                        nc.vector.tensor_copy(o65t[:], o_ps[h][:])
                        o65_all[(h, jq)] = o65t

                # normalization for this head pair (overlaps the next pair's
                # attention)
                for h in heads:
                    for jq in range(NT):
                        o65t = o65_all[(h, jq)]
                        lnd = rcpool.tile([65, 512], F32, tag="lnd", name=f"ln{h}{jq}")
                        nc.scalar.activation(
                            lnd[64:65, :],
                            o65t[64:65, :].bitcast(F32),
                            mybir.ActivationFunctionType.Ln,
                        )
                        rc = rcpool.tile([65, 512], F32R, tag="rc", name=f"rc{h}{jq}")
                        nc.scalar.activation(
                            rc[64:65, :],
                            lnd[64:65, :],
                            mybir.ActivationFunctionType.Exp,
                            scale=-1.0,
                        )
                        bc_ps = misc_ps.tile([64, 512], F32, tag="misc", name=f"bc{h}{jq}")
                        nc.tensor.matmul(
                            bc_ps[:],
                            ones65[64:65, :],
                            rc[64:65, :],
                            start=True,
                            stop=True,
                            tile_position=(64, 0),
                        )
                        nc.vector.tensor_tensor(
                            opk[(h % 2) * 64 : (h % 2) * 64 + 64, h // 2, bass.ts(jq, 512)],
                            o65t[0:64, :].bitcast(F32),
                            bc_ps[:],
                            mybir.AluOpType.mult,
                        )
            attn_ctx.close()
            # ---------------- phase D: Wo projection ----------------
            with tc.tile_pool(name="wops", bufs=6, space="PSUM") as wop:
                for t in range(T // 128):
                    for nn in range(2):
                        wps = wop.tile([128, 512], F32, tag="wo", name="wps")
                        for k in range(2):
                            nc.tensor.matmul(
                                wps[:],
                                opk[:, k, bass.ts(t, 128)],
                                wo_sb[:, k, bass.ts(nn, 512)],
                                start=(k == 0),
                                stop=(k == 1),
                            )
                        y_sb = ypool.tile([128, 512], F32, tag="y", name="ysb")
                        if (t + nn) % 2 == 0:
                            nc.vector.tensor_copy(y_sb[:], wps[:])
                        else:
                            nc.scalar.copy(y_sb[:], wps[:])
                        nc.sync.dma_start(
                            y_d[bass.ts(t, 128), bass.ts(nn, 512)], y_sb[:]
                        )

    _split_excess_waits(nc)
    return nc


_NC_CACHE = None


def _get_nc():
    global _NC_CACHE
    if _NC_CACHE is None:
        _NC_CACHE = _build()
    return _NC_CACHE


def _host_prep(x, cos, sin, Wq, Wk, Wv, Wo):
    cos2 = np.asarray(cos, np.float32).reshape(T, HALF)  # [T, 32]
    sin2 = np.asarray(sin, np.float32).reshape(T, HALF)
    atab = np.tile(cos2.T, (4, 1))  # [128, T]
    btab = np.tile(np.vstack([sin2.T, -sin2.T]), (2, 1))  # [128, T]
    idx = np.arange(128)
    pswap = np.zeros((128, 128), np.float32)
    pswap[idx ^ 32, idx] = 1.0
    k_i = np.arange(128)[:, None]
    q_i = np.arange(128)[None, :]
    trib = np.where(k_i > q_i, np.float32(NEG), np.float32(0.0))
    import ml_dtypes

    trib = trib.astype(ml_dtypes.bfloat16)
    identb = np.eye(128, dtype=ml_dtypes.bfloat16)
    identr = np.eye(64, dtype=np.float32)
    ones65 = np.ones((65, 64), np.float32)
    ones16 = np.ones((128, 16), np.float32)
    zeros64 = np.zeros((64, T), np.float32)

    in_maps = []
    for core in range(8):
        b, g = core // 4, core % 4
        xt = np.ascontiguousarray(np.asarray(x[b], np.float32).T)  # [C, T]
        wproj = np.ascontiguousarray(
            np.concatenate(
                [
                    Wq[:, g * FQ : (g + 1) * FQ],
                    Wk[:, g * D : (g + 1) * D],
                    Wv[:, g * D : (g + 1) * D],
                ],
                axis=1,
            ).astype(np.float32)
        )
        wo = np.ascontiguousarray(Wo[g * FQ : (g + 1) * FQ, :].astype(np.float32))
        in_maps.append(
            {
                "xt": xt,
                "wproj": wproj,
                "wo": wo,
                "atab": atab,
                "btab": btab,
                "pswap": pswap,
                "trib": trib,
                "identb": identb,
                "identr": identr,
                "ones65": ones65,
                "ones16": ones16,
                "zeros64": zeros64,
            }
        )
    return in_maps


def kernel(x, cos, sin, Wq, Wk, Wv, Wo, _want_trace=False, _trace_kwargs=None):
    nc = _get_nc()
    in_maps = _host_prep(x, cos, sin, Wq, Wk, Wv, Wo)
    kw = {}
    if _want_trace:
        kw = dict(trace=True, **(_trace_kwargs or {}))
    res = run_bass_kernel_spmd(nc, in_maps, list(range(8)), **kw)
    y = np.zeros((B, T, C), np.float32)
    for core in range(8):
        b = core // 4
        y[b] += res.results[core]["y"]
    if _want_trace:
        kernel.last_result = res
    return y


# revision 25
# speedup vs baseline: 1.2236x; 1.0013x over previous
"""Causal self-attention (GQA + RoPE) Trainium2 Bass kernel.

Sharding: 8 cores = batch(2) x kv-group(4). Each core computes its batch's
4 q-heads / 1 kv-head and a row-shard of the Wo projection; the 4 partial
outputs per batch are summed on host (all-reduce replacement).

Self-contained: hardcodes all shapes from the problem spec.
"""

import numpy as np

import concourse.bass as bass
import concourse.mybir as mybir
from concourse.tile import TileContext
from concourse.bass_utils import run_bass_kernel_spmd

F32 = mybir.dt.float32
F32R = mybir.dt.float32r
BF16 = mybir.dt.bfloat16

B, T, C = 2, 2048, 1024
H, HKV, D = 16, 4, 64
HALF = D // 2  # 32
GQ = H // HKV  # 4 q heads per group
FQ = GQ * D    # 256 q features per group
FPROJ = FQ + 2 * D  # 384: q(256) + k(64) + v(64)
NT = T // 512  # 4 column blocks of 512
KT = C // 128  # 8 contraction tiles
MT = FPROJ // 128  # 3 output row tiles (q01, q23, kv)
NEG = -1.0e9


def _split_excess_waits(nc, max_waits=1):
    """walrus here encodes at most one sync-wait per instruction; hoist the
    rest into standalone EventSemaphore instructions (raw-bass encoding)."""
    n = 0
    for fn in nc.m.functions:
        for bb in fn.blocks:
            new = []
            changed = False
            for inst in bb.instructions:
                si = inst.sync_info
                if si is not None and len(si.on_wait) > max_waits:
                    waits = list(si.on_wait)
                    for j, w in enumerate(waits[max_waits:]):
                        ev = mybir.InstEventSemaphore(
                            name=f"{inst.name}-ws{j}",
                            engine=inst.engine,
                            ins=[],
                            outs=[],
                            sync_info=mybir.SyncInfo(on_wait=[w], on_update=[]),
                        )
                        new.append(ev)
                        n += 1
                    inst.sync_info = mybir.SyncInfo(
                        on_wait=waits[:max_waits], on_update=list(si.on_update)
                    )
                    changed = True
                new.append(inst)
            if changed:
                bb.instructions = new
    return n


def _build():
    nc = bass.Bass()
    xt_d = nc.dram_tensor("xt", [C, T], F32, kind="ExternalInput")
    wproj_d = nc.dram_tensor("wproj", [C, FPROJ], F32, kind="ExternalInput")
    wo_d = nc.dram_tensor("wo", [FQ, C], F32, kind="ExternalInput")
    atab_d = nc.dram_tensor("atab", [128, T], F32, kind="ExternalInput")
    btab_d = nc.dram_tensor("btab", [128, T], F32, kind="ExternalInput")
    pswap_d = nc.dram_tensor("pswap", [128, 128], F32, kind="ExternalInput")
    trib_d = nc.dram_tensor("trib", [128, 128], BF16, kind="ExternalInput")
    identb_d = nc.dram_tensor("identb", [128, 128], BF16, kind="ExternalInput")
    identr_d = nc.dram_tensor("identr", [64, 64], F32, kind="ExternalInput")
    ones65_d = nc.dram_tensor("ones65", [65, 64], F32, kind="ExternalInput")
    ones16_d = nc.dram_tensor("ones16", [128, 16], F32, kind="ExternalInput")
    zeros_d = nc.dram_tensor("zeros64", [64, T], F32, kind="ExternalInput")
    y_d = nc.dram_tensor("y", [T, C], F32, kind="ExternalOutput")

    with TileContext(nc) as tc:
        from contextlib import ExitStack

        with ExitStack() as ctx:
            const = ctx.enter_context(tc.tile_pool(name="const", bufs=1))
            pers = ctx.enter_context(tc.tile_pool(name="pers", bufs=1))
            # --- constants ---
            wproj_sb = const.tile([128, KT, FPROJ], F32R)
            wproj_r = wproj_d.rearrange("(ko p) f -> p ko f", p=128).bitcast(F32R)
            wo_sb = const.tile([128, 2, C], F32R)
            atab = const.tile([128, T], F32)
            btab = const.tile([128, T], F32)
            pswap = const.tile([128, 128], F32R)
            trib = const.tile([128, 128], BF16)
            identb = const.tile([128, 128], BF16)
            identr = const.tile([128, 64], F32R)
            ones65 = const.tile([65, 64], F32R)

            # --- persistent activations ---
            qr = [pers.tile([128, T], F32R, name=f"qr{i}") for i in range(2)]
            # k^T zero-padded to 128 contraction rows: kr0 = [k; 0] for even
            # heads, kr1 = [0; k] for odd heads -> S matmuls engage the full
            # PE array (HAM stays warm) while the zero half kills the other
            # head's q rows exactly.
            kr0 = pers.tile([128, T], F32R)
            kr1 = pers.tile([128, T], F32R)
            vsb = pers.tile([128, T // 128, 65], F32R)  # v natural + ones col
            opk = pers.tile([128, 2, T], F32R)  # packed normalized O^T for Wo

            misc_ps = ctx.enter_context(
                tc.tile_pool(name="miscps", bufs=2, space="PSUM")
            )

            # ---------------- phase A+B: projections + RoPE ----------------
            with tc.tile_pool(name="xt", bufs=32) as xpool, tc.tile_pool(
                name="tmp", bufs=3
            ) as tmp, tc.tile_pool(name="paps", bufs=3, space="PSUM") as pa_ps:
                # plain copy of kv proj (v^T in rows 64:); dead after phase B
                kvp = tmp.tile([128, T], F32R, tag="kvp", name="kvp")
                xts = {}
                for n in range(NT):
                    for k in range(KT):
                        if n == 0:
                            nc.sync.dma_start(wproj_sb[:, k], wproj_r[:, k])
                        t = xpool.tile([128, 512], F32R, tag="x", name=f"x{k}_{n}")
                        nc.sync.dma_start(
                            t[:], xt_d[bass.ts(k, 128), bass.ts(n, 512)].bitcast(F32R)
                        )
                        xts[(k, n)] = t
                    if n == 0:
                        # small constants (cheap), then RoPE tables
                        nc.sync.dma_start(pswap[:], pswap_d[:].bitcast(F32R))
                        nc.sync.dma_start(identr[64:128, :], identr_d[:].bitcast(F32R))
                        nc.sync.dma_start(trib[:], trib_d[:])
                        nc.sync.dma_start(identb[:], identb_d[:])
                        nc.sync.dma_start(ones65[:], ones65_d[:].bitcast(F32R))
                        nc.sync.dma_start(atab[:], atab_d[:])
                        nc.sync.dma_start(btab[:], btab_d[:])
                    if n == 1:
                        nc.sync.dma_start(kr0[64:128, :], zeros_d[:].bitcast(F32R))
                        nc.sync.dma_start(kr1[0:64, :], zeros_d[:].bitcast(F32R))
                        nc.sync.dma_start(vsb[:, :, 64], ones16_d[:].bitcast(F32R))
                        nc.sync.dma_start(
                            wo_sb[:],
                            wo_d.rearrange("(ko p) c -> p ko c", p=128).bitcast(F32R),
                        )

                for n in range(NT):
                    for m in range(MT):
                        ps = pa_ps.tile([128, 512], F32, tag="proj")
                        for k in range(KT):
                            nc.tensor.matmul(
                                ps[:],
                                wproj_sb[:, k, bass.ts(m, 128)],
                                xts[(k, n)][:],
                                start=(k == 0),
                                stop=(k == KT - 1),
                            )
                        # RoPE on this [128, 512] block
                        rows = 128 if m < 2 else 64
                        if m == 2:
                            plain = kvp[:, bass.ts(n, 512)]
                        else:
                            qt_t = tmp.tile([128, 512], F32R, tag="qtmp", name="qtmp")
                            plain = qt_t[:]
                        nc.vector.tensor_copy(plain, ps[:])
                        qsw = pa_ps.tile([128, 512], F32, tag="qsw")
                        nc.tensor.matmul(
                            qsw[0:rows],
                            pswap[0:rows, 0:rows],
                            plain[0:rows],
                            start=True,
                            stop=True,
                        )
                        t1 = tmp.tile([128, 512], F32, tag="t1")
                        nc.vector.tensor_tensor(
                            t1[0:rows],
                            ps[0:rows],
                            atab[0:rows, bass.ts(n, 512)],
                            mybir.AluOpType.mult,
                        )
                        t2 = tmp.tile([128, 512], F32, tag="t2")
                        nc.vector.tensor_tensor(
                            t2[0:rows],
                            qsw[0:rows],
                            btab[0:rows, bass.ts(n, 512)],
                            mybir.AluOpType.mult,
                        )
                        dest = qr[m] if m < 2 else kr0
                        nc.gpsimd.tensor_tensor(
                            dest[0:rows, bass.ts(n, 512)],
                            t1[0:rows],
                            t2[0:rows],
                            mybir.AluOpType.add,
                        )
                        if m == 2:
                            # duplicate k^T into kr1 rows 64:128
                            nc.vector.tensor_copy(
                                kr1[64:128, bass.ts(n, 512)],
                                kr0[0:64, bass.ts(n, 512)],
                            )
                            # v^T -> v natural (PE transpose per 128-token block)
                            for tt in range(4 * n, 4 * n + 4):
                                vt_ps = misc_ps.tile([128, 64], F32, tag="misc")
                                nc.tensor.transpose(
                                    vt_ps[:],
                                    kvp[64:128, bass.ts(tt, 128)].bitcast(F32),
                                    identr[64:128, :].bitcast(F32),
                                )
                                nc.vector.tensor_copy(vsb[:, tt, 0:64], vt_ps[:])

            # ---------------- phase C: attention ----------------
            attn_ctx = ExitStack()
            spool = attn_ctx.enter_context(
                tc.tile_pool(name="attnps", bufs=2, space="PSUM")
            )
            opool = spool  # same pool, separate tags
            ppool = ctx.enter_context(tc.tile_pool(name="pp", bufs=4))
            o65pool = ctx.enter_context(tc.tile_pool(name="o65p", bufs=12))
            rcpool = ctx.enter_context(tc.tile_pool(name="rc", bufs=2))
            ypool = ctx.enter_context(tc.tile_pool(name="yp", bufs=4))
            o65_all = {}
            pending_c2 = []

            def emit_c2(h, jq):
                o65t = o65_all[(h, jq)]
                lnd = rcpool.tile([65, 512], F32, tag="lnd", name=f"ln{h}{jq}")
                nc.scalar.activation(
                    lnd[64:65, :],
                    o65t[64:65, :].bitcast(F32),
                    mybir.ActivationFunctionType.Ln,
                )
                rc = rcpool.tile([65, 512], F32R, tag="rc", name=f"rc{h}{jq}")
                nc.scalar.activation(
                    rc[64:65, :],
                    lnd[64:65, :],
                    mybir.ActivationFunctionType.Exp,
                    scale=-1.0,
                )
                bc_ps = misc_ps.tile([64, 512], F32, tag="misc", name=f"bc{h}{jq}")
                nc.tensor.matmul(
                    bc_ps[:],
                    ones65[64:65, :],
                    rc[64:65, :],
                    start=True,
                    stop=True,
                    tile_position=(64, 0),
                )
                nc.vector.tensor_tensor(
                    opk[(h % 2) * 64 : (h % 2) * 64 + 64, h // 2, bass.ts(jq, 512)],
                    o65t[0:64, :].bitcast(F32),
                    bc_ps[:],
                    mybir.AluOpType.mult,
                )

            for hp in range(2):
                qtile = qr[hp]
                heads = (2 * hp, 2 * hp + 1)
                for jq in range(NT):
                    o_ps = {
                        h: opool.tile([65, 512], F32, tag="o", name=f"o{h}")
                        for h in heads
                    }
                    nkb = 4 * (jq + 1)
                    for ksb in range(nkb // 2):
                        regions = []
                        for jk in range(2):
                            kb = 2 * ksb + jk
                            j = kb - 4 * jq
                            col0 = max(j, 0) * 128
                            regions.append((jk, col0, kb))
                        s_ps = {}
                        p_sb = {}
                        for h in heads:
                            s_ps[h] = spool.tile([128, 1024], F32, tag="s", name=f"s{h}")
                            p_sb[h] = ppool.tile([128, 1024], F32R, tag="p", name=f"pb{h}")
                        for jk, col0, kb in regions:
                            for h in heads:
                                krt = kr0 if h % 2 == 0 else kr1
                                nc.tensor.matmul(
                                    s_ps[h][:, jk * 512 + col0 : jk * 512 + 512],
                                    krt[:, bass.ts(kb, 128)],
                                    qtile[:, jq * 512 + col0 : jq * 512 + 512],
                                    start=True,
                                    stop=(kb < 4 * jq),
                                )
                        for h in heads:
                            for jk, col0, kb in regions:
                                if kb - 4 * jq >= 0:
                                    nc.tensor.matmul(
                                        s_ps[h][:, jk * 512 + col0 : jk * 512 + col0 + 128],
                                        identb[:],
                                        trib[:],
                                        start=False,
                                        stop=True,
                                    )
                        for h in heads:
                            if regions[0][1] == 0 and regions[1][1] == 0:
                                nc.scalar.activation(
                                    p_sb[h][:],
                                    s_ps[h][:],
                                    mybir.ActivationFunctionType.Exp,
                                    scale=0.125,
                                )
                            else:
                                for jk, col0, kb in regions:
                                    nc.scalar.activation(
                                        p_sb[h][:, jk * 512 + col0 : jk * 512 + 512],
                                        s_ps[h][:, jk * 512 + col0 : jk * 512 + 512],
                                        mybir.ActivationFunctionType.Exp,
                                        scale=0.125,
                                    )
                        for h in heads:
                            for jk, col0, kb in regions:
                                nc.tensor.matmul(
                                    o_ps[h][:, col0:512],
                                    vsb[:, kb, :],
                                    p_sb[h][:, jk * 512 + col0 : jk * 512 + 512],
                                    start=(kb == 0),
                                    stop=(kb == nkb - 1),
                                )
                    for h in heads:
                        o65t = o65pool.tile(
                            [65, 512], F32R, tag="o65", name=f"o65_{h}_{jq}"
                        )
                        nc.vector.tensor_copy(o65t[:], o_ps[h][:])
                        o65_all[(h, jq)] = o65t
                    if hp == 1:
                        for item in pending_c2[:2]:
                            emit_c2(*item)
                        del pending_c2[:2]

                # queue normalization work; drained interleaved with the
                # next head-pair's attention so it fills engine gaps
                for h in heads:
                    for jq in range(NT):
                        pending_c2.append((h, jq))
                if hp == 1:
                    for item in pending_c2:
                        emit_c2(*item)
                    pending_c2.clear()

            attn_ctx.close()
            # ---------------- phase D: Wo projection ----------------
            with tc.tile_pool(name="wops", bufs=6, space="PSUM") as wop:
                for t in range(T // 128):
                    for nn in range(2):
                        wps = wop.tile([128, 512], F32, tag="wo", name="wps")
                        for k in range(2):
                            nc.tensor.matmul(
                                wps[:],
                                opk[:, k, bass.ts(t, 128)],
                                wo_sb[:, k, bass.ts(nn, 512)],
                                start=(k == 0),
                                stop=(k == 1),
                            )
                        y_sb = ypool.tile([128, 512], F32, tag="y", name="ysb")
                        if (t + nn) % 2 == 0:
                            nc.vector.tensor_copy(y_sb[:], wps[:])
                        else:
                            nc.scalar.copy(y_sb[:], wps[:])
                        nc.sync.dma_start(
                            y_d[bass.ts(t, 128), bass.ts(nn, 512)], y_sb[:]
                        )

    _split_excess_waits(nc)
    return nc


_NC_CACHE = None


def _get_nc():
    global _NC_CACHE
    if _NC_CACHE is None:
        _NC_CACHE = _build()
    return _NC_CACHE


def _host_prep(x, cos, sin, Wq, Wk, Wv, Wo):
    cos2 = np.asarray(cos, np.float32).reshape(T, HALF)  # [T, 32]
    sin2 = np.asarray(sin, np.float32).reshape(T, HALF)
    atab = np.tile(cos2.T, (4, 1))  # [128, T]
    btab = np.tile(np.vstack([sin2.T, -sin2.T]), (2, 1))  # [128, T]
    idx = np.arange(128)
    pswap = np.zeros((128, 128), np.float32)
    pswap[idx ^ 32, idx] = 1.0
    k_i = np.arange(128)[:, None]
    q_i = np.arange(128)[None, :]
    trib = np.where(k_i > q_i, np.float32(NEG), np.float32(0.0))
    import ml_dtypes

    trib = trib.astype(ml_dtypes.bfloat16)
    identb = np.eye(128, dtype=ml_dtypes.bfloat16)
    identr = np.eye(64, dtype=np.float32)
    ones65 = np.ones((65, 64), np.float32)
    ones16 = np.ones((128, 16), np.float32)
    zeros64 = np.zeros((64, T), np.float32)

    in_maps = []
    for core in range(8):
        b, g = core // 4, core % 4
        xt = np.ascontiguousarray(np.asarray(x[b], np.float32).T)  # [C, T]
        wproj = np.ascontiguousarray(
            np.concatenate(
                [
                    Wq[:, g * FQ : (g + 1) * FQ],
                    Wk[:, g * D : (g + 1) * D],
                    Wv[:, g * D : (g + 1) * D],
                ],
                axis=1,
            ).astype(np.float32)
        )
        wo = np.ascontiguousarray(Wo[g * FQ : (g + 1) * FQ, :].astype(np.float32))
        in_maps.append(
            {
                "xt": xt,
                "wproj": wproj,
                "wo": wo,
                "atab": atab,
                "btab": btab,
                "pswap": pswap,
                "trib": trib,
                "identb": identb,
                "identr": identr,
                "ones65": ones65,
                "ones16": ones16,
                "zeros64": zeros64,
            }
        )
    return in_maps


def kernel(x, cos, sin, Wq, Wk, Wv, Wo, _want_trace=False, _trace_kwargs=None):
    nc = _get_nc()
    in_maps = _host_prep(x, cos, sin, Wq, Wk, Wv, Wo)
    kw = {}
    if _want_trace:
        kw = dict(trace=True, **(_trace_kwargs or {}))
    res = run_bass_kernel_spmd(nc, in_maps, list(range(8)), **kw)
    y = np.zeros((B, T, C), np.float32)
    for core in range(8):
        b = core // 4
        y[b] += res.results[core]["y"]
    if _want_trace:
        kernel.last_result = res
    return y


# revision 28
# speedup vs baseline: 1.4422x; 1.1786x over previous
"""Causal self-attention (GQA + RoPE) Trainium2 Bass kernel.

Sharding: 8 cores = batch(2) x kv-group(4). Each core computes its batch's
4 q-heads / 1 kv-head and a row-shard of the Wo projection; the 4 partial
outputs per batch are summed on host (all-reduce replacement).

Self-contained: hardcodes all shapes from the problem spec.
"""

import numpy as np

import concourse.bass as bass
import concourse.mybir as mybir
from concourse.tile import TileContext
from concourse.bass_utils import run_bass_kernel_spmd

F32 = mybir.dt.float32
F32R = mybir.dt.float32r
BF16 = mybir.dt.bfloat16

B, T, C = 2, 2048, 1024
H, HKV, D = 16, 4, 64
HALF = D // 2  # 32
GQ = H // HKV  # 4 q heads per group
FQ = GQ * D    # 256 q features per group
FPROJ = FQ + 2 * D  # 384: q(256) + k(64) + v(64)
NT = T // 512  # 4 column blocks of 512
KT = C // 128  # 8 contraction tiles
MT = FPROJ // 128  # 3 output row tiles (q01, q23, kv)
NEG = -1.0e9


def _split_excess_waits(nc, max_waits=1):
    """walrus here encodes at most one sync-wait per instruction; hoist the
    rest into standalone EventSemaphore instructions (raw-bass encoding)."""
    n = 0
    for fn in nc.m.functions:
        for bb in fn.blocks:
            new = []
            changed = False
            for inst in bb.instructions:
                si = inst.sync_info
                if si is not None and len(si.on_wait) > max_waits:
                    waits = list(si.on_wait)
                    for j, w in enumerate(waits[max_waits:]):
                        ev = mybir.InstEventSemaphore(
                            name=f"{inst.name}-ws{j}",
                            engine=inst.engine,
                            ins=[],
                            outs=[],
                            sync_info=mybir.SyncInfo(on_wait=[w], on_update=[]),
                        )
                        new.append(ev)
                        n += 1
                    inst.sync_info = mybir.SyncInfo(
                        on_wait=waits[:max_waits], on_update=list(si.on_update)
                    )
                    changed = True
                new.append(inst)
            if changed:
                bb.instructions = new
    return n


def _build():
    nc = bass.Bass()
    xt_d = nc.dram_tensor("xt", [C, T], F32, kind="ExternalInput")
    wproj_d = nc.dram_tensor("wproj", [C, FPROJ], F32, kind="ExternalInput")
    wo_d = nc.dram_tensor("wo", [FQ, C], F32, kind="ExternalInput")
    atab_d = nc.dram_tensor("atab", [128, T], F32, kind="ExternalInput")
    btab_d = nc.dram_tensor("btab", [128, T], F32, kind="ExternalInput")
    pswap_d = nc.dram_tensor("pswap", [128, 128], F32, kind="ExternalInput")
    trib_d = nc.dram_tensor("trib", [128, 128], BF16, kind="ExternalInput")
    identb_d = nc.dram_tensor("identb", [128, 128], BF16, kind="ExternalInput")
    identr_d = nc.dram_tensor("identr", [64, 64], F32, kind="ExternalInput")
    ones65_d = nc.dram_tensor("ones65", [65, 64], F32, kind="ExternalInput")
    ones16_d = nc.dram_tensor("ones16", [128, 16], F32, kind="ExternalInput")
    zeros_d = nc.dram_tensor("zeros64", [64, T], F32, kind="ExternalInput")
    y_d = nc.dram_tensor("y", [T, C], F32, kind="ExternalOutput")

    with TileContext(nc) as tc:
        from contextlib import ExitStack

        with ExitStack() as ctx:
            const = ctx.enter_context(tc.tile_pool(name="const", bufs=1))
            pers = ctx.enter_context(tc.tile_pool(name="pers", bufs=1))
            # --- constants ---
            wproj_sb = const.tile([128, KT, FPROJ], F32R)
            wproj_r = wproj_d.rearrange("(ko p) f -> p ko f", p=128).bitcast(F32R)
            wo_sb = const.tile([128, 2, C], F32R)
            atab = const.tile([128, T], F32)
            btab = const.tile([128, T], F32)
            pswap = const.tile([128, 128], F32R)
            trib = const.tile([128, 128], BF16)
            identb = const.tile([128, 128], BF16)
            identr = const.tile([128, 64], F32R)
            ones65 = const.tile([65, 64], F32R)

            # --- persistent activations ---
            qr = [pers.tile([128, T], F32R, name=f"qr{i}") for i in range(2)]
            # k^T zero-padded to 128 contraction rows: kr0 = [k; 0] for even
            # heads, kr1 = [0; k] for odd heads -> S matmuls engage the full
            # PE array (HAM stays warm) while the zero half kills the other
            # head's q rows exactly.
            kr0 = pers.tile([128, T], F32R)
            kr1 = pers.tile([128, T], F32R)
            vsb = pers.tile([128, T // 128, 65], F32R)  # v natural + ones col
            opk = pers.tile([128, 2, T], F32R)  # packed normalized O^T for Wo

            # ---------------- phase A+B: projections + RoPE ----------------
            with tc.tile_pool(name="xt", bufs=32) as xpool, tc.tile_pool(
                name="tmp", bufs=4
            ) as tmp, tc.tile_pool(name="paps", bufs=4, space="PSUM") as pa_ps:
                # plain copy of kv proj (v^T in rows 64:); dead after phase B
                kvp = tmp.tile([128, T], F32R, tag="kvp", name="kvp", bufs=1)
                xts = {}
                for n in range(NT):
                    for k in range(KT):
                        if n == 0:
                            nc.sync.dma_start(wproj_sb[:, k], wproj_r[:, k])
                        t = xpool.tile([128, 512], F32R, tag="x", name=f"x{k}_{n}")
                        nc.sync.dma_start(
                            t[:], xt_d[bass.ts(k, 128), bass.ts(n, 512)].bitcast(F32R)
                        )
                        xts[(k, n)] = t
                    if n == 0:
                        # small constants (cheap), then RoPE tables
                        nc.sync.dma_start(pswap[:], pswap_d[:].bitcast(F32R))
                        nc.sync.dma_start(identr[64:128, :], identr_d[:].bitcast(F32R))
                        nc.sync.dma_start(trib[:], trib_d[:])
                        nc.sync.dma_start(identb[:], identb_d[:])
                        nc.sync.dma_start(ones65[:], ones65_d[:].bitcast(F32R))
                        nc.sync.dma_start(atab[:], atab_d[:])
                        nc.sync.dma_start(btab[:], btab_d[:])
                    if n == 1:
                        nc.sync.dma_start(kr0[64:128, :], zeros_d[:].bitcast(F32R))
                        nc.sync.dma_start(kr1[0:64, :], zeros_d[:].bitcast(F32R))
                        nc.sync.dma_start(vsb[:, :, 64], ones16_d[:].bitcast(F32R))
                        nc.sync.dma_start(
                            wo_sb[:],
                            wo_d.rearrange("(ko p) c -> p ko c", p=128).bitcast(F32R),
                        )

                for n in range(NT):
                    for m in range(MT):
                        ps = pa_ps.tile([128, 512], F32, tag="proj")
                        for k in range(KT):
                            nc.tensor.matmul(
                                ps[:],
                                wproj_sb[:, k, bass.ts(m, 128)],
                                xts[(k, n)][:],
                                start=(k == 0),
                                stop=(k == KT - 1),
                            )
                        # RoPE on this [128, 512] block
                        rows = 128 if m < 2 else 64
                        if m == 2:
                            plain = kvp[:, bass.ts(n, 512)]
                        else:
                            qt_t = tmp.tile([128, 512], F32R, tag="qtmp", name="qtmp")
                            plain = qt_t[:]
                        nc.vector.tensor_copy(plain, ps[:])
                        qsw = pa_ps.tile([128, 512], F32, tag="qsw")
                        nc.tensor.matmul(
                            qsw[0:rows],
                            pswap[0:rows, 0:rows],
                            plain[0:rows],
                            start=True,
                            stop=True,
                        )
                        t1 = tmp.tile([128, 512], F32, tag="t1")
                        nc.vector.tensor_tensor(
                            t1[0:rows],
                            ps[0:rows],
                            atab[0:rows, bass.ts(n, 512)],
                            mybir.AluOpType.mult,
                        )
                        t2 = tmp.tile([128, 512], F32, tag="t2")
                        nc.vector.tensor_tensor(
                            t2[0:rows],
                            qsw[0:rows],
                            btab[0:rows, bass.ts(n, 512)],
                            mybir.AluOpType.mult,
                        )
                        dest = qr[m] if m < 2 else kr0
                        nc.gpsimd.tensor_tensor(
                            dest[0:rows, bass.ts(n, 512)],
                            t1[0:rows],
                            t2[0:rows],
                            mybir.AluOpType.add,
                        )
                        if m == 2:
                            # duplicate k^T into kr1 rows 64:128
                            nc.vector.tensor_copy(
                                kr1[64:128, bass.ts(n, 512)],
                                kr0[0:64, bass.ts(n, 512)],
                            )
                            # v^T -> v natural (PE transpose per 128-token block)
                            for tt in range(4 * n, 4 * n + 4):
                                vt_ps = pa_ps.tile([128, 64], F32, tag="qsw")
                                nc.tensor.transpose(
                                    vt_ps[:],
                                    kvp[64:128, bass.ts(tt, 128)].bitcast(F32),
                                    identr[64:128, :].bitcast(F32),
                                )
                                nc.vector.tensor_copy(vsb[:, tt, 0:64], vt_ps[:])

            # ---------------- phase C: attention ----------------
            misc_ps = ctx.enter_context(
                tc.tile_pool(name="miscps", bufs=2, space="PSUM")
            )
            attn_ctx = ExitStack()
            spool = attn_ctx.enter_context(
                tc.tile_pool(name="attnps", bufs=2, space="PSUM")
            )
            opool = spool  # same pool, separate tags
            ppool = ctx.enter_context(tc.tile_pool(name="pp", bufs=4))
            o65pool = ctx.enter_context(tc.tile_pool(name="o65p", bufs=12))
            rcpool = ctx.enter_context(tc.tile_pool(name="rc", bufs=2))
            ypool = ctx.enter_context(tc.tile_pool(name="yp", bufs=4))
            o65_all = {}
            pending_c2 = []

            def emit_c2(h, jq):
                o65t = o65_all[(h, jq)]
                lnd = rcpool.tile([65, 512], F32, tag="lnd", name=f"ln{h}{jq}")
                nc.scalar.activation(
                    lnd[64:65, :],
                    o65t[64:65, :].bitcast(F32),
                    mybir.ActivationFunctionType.Ln,
                )
                rc = rcpool.tile([65, 512], F32R, tag="rc", name=f"rc{h}{jq}")
                nc.scalar.activation(
                    rc[64:65, :],
                    lnd[64:65, :],
                    mybir.ActivationFunctionType.Exp,
                    scale=-1.0,
                )
                bc_ps = misc_ps.tile([64, 512], F32, tag="misc", name=f"bc{h}{jq}")
                nc.tensor.matmul(
                    bc_ps[:],
                    ones65[64:65, :],
                    rc[64:65, :],
                    start=True,
                    stop=True,
                    tile_position=(64, 0),
                )
                nc.vector.tensor_tensor(
                    opk[(h % 2) * 64 : (h % 2) * 64 + 64, h // 2, bass.ts(jq, 512)],
                    o65t[0:64, :].bitcast(F32),
                    bc_ps[:],
                    mybir.AluOpType.mult,
                )

            for hp in range(2):
                qtile = qr[hp]
                heads = (2 * hp, 2 * hp + 1)
                for jq in range(NT):
                    o_ps = {
                        h: opool.tile([65, 512], F32, tag="o", name=f"o{h}")
                        for h in heads
                    }
                    nkb = 4 * (jq + 1)
                    for ksb in range(nkb // 2):
                        regions = []
                        for jk in range(2):
                            kb = 2 * ksb + jk
                            j = kb - 4 * jq
                            col0 = max(j, 0) * 128
                            regions.append((jk, col0, kb))
                        s_ps = {}
                        p_sb = {}
                        for h in heads:
                            s_ps[h] = spool.tile([128, 1024], F32, tag="s", name=f"s{h}")
                            p_sb[h] = ppool.tile([128, 1024], F32R, tag="p", name=f"pb{h}")
                        for jk, col0, kb in regions:
                            for h in heads:
                                krt = kr0 if h % 2 == 0 else kr1
                                nc.tensor.matmul(
                                    s_ps[h][:, jk * 512 + col0 : jk * 512 + 512],
                                    krt[:, bass.ts(kb, 128)],
                                    qtile[:, jq * 512 + col0 : jq * 512 + 512],
                                    start=True,
                                    stop=(kb < 4 * jq),
                                )
                        for h in heads:
                            for jk, col0, kb in regions:
                                if kb - 4 * jq >= 0:
                                    nc.tensor.matmul(
                                        s_ps[h][:, jk * 512 + col0 : jk * 512 + col0 + 128],
                                        identb[:],
                                        trib[:],
                                        start=False,
                                        stop=True,
                                    )
                        for h in heads:
                            if regions[0][1] == 0 and regions[1][1] == 0:
                                nc.scalar.activation(
                                    p_sb[h][:],
                                    s_ps[h][:],
                                    mybir.ActivationFunctionType.Exp,
                                    scale=0.125,
                                )
                            else:
                                for jk, col0, kb in regions:
                                    nc.scalar.activation(
                                        p_sb[h][:, jk * 512 + col0 : jk * 512 + 512],
                                        s_ps[h][:, jk * 512 + col0 : jk * 512 + 512],
                                        mybir.ActivationFunctionType.Exp,
                                        scale=0.125,
                                    )
                        for h in heads:
                            for jk, col0, kb in regions:
                                nc.tensor.matmul(
                                    o_ps[h][:, col0:512],
                                    vsb[:, kb, :],
                                    p_sb[h][:, jk * 512 + col0 : jk * 512 + 512],
                                    start=(kb == 0),
                                    stop=(kb == nkb - 1),
                                )
                    for h in heads:
                        o65t = o65pool.tile(
                            [65, 512], F32R, tag="o65", name=f"o65_{h}_{jq}"
                        )
                        nc.vector.tensor_copy(o65t[:], o_ps[h][:])
                        o65_all[(h, jq)] = o65t
                    if hp == 1:
                        for item in pending_c2[:2]:
                            emit_c2(*item)
                        del pending_c2[:2]

                # queue normalization work; drained interleaved with the
                # next head-pair's attention so it fills engine gaps
                for h in heads:
                    for jq in range(NT):
                        pending_c2.append((h, jq))
                if hp == 1:
                    for item in pending_c2:
                        emit_c2(*item)
                    pending_c2.clear()

            attn_ctx.close()
            # ---------------- phase D: Wo projection ----------------
            with tc.tile_pool(name="wops", bufs=6, space="PSUM") as wop:
                for t in range(T // 128):
                    for nn in range(2):
                        wps = wop.tile([128, 512], F32, tag="wo", name="wps")
                        for k in range(2):
                            nc.tensor.matmul(
                                wps[:],
                                opk[:, k, bass.ts(t, 128)],
                                wo_sb[:, k, bass.ts(nn, 512)],
                                start=(k == 0),
                                stop=(k == 1),
                            )
                        y_sb = ypool.tile([128, 512], F32, tag="y", name="ysb")
                        if (t + nn) % 2 == 0:
                            nc.vector.tensor_copy(y_sb[:], wps[:])
                        else:
                            nc.scalar.copy(y_sb[:], wps[:])
                        nc.sync.dma_start(
                            y_d[bass.ts(t, 128), bass.ts(nn, 512)], y_sb[:]
                        )

    _split_excess_waits(nc)
    return nc


_NC_CACHE = None


def _get_nc():
    global _NC_CACHE
    if _NC_CACHE is None:
        _NC_CACHE = _build()
    return _NC_CACHE


def _host_prep(x, cos, sin, Wq, Wk, Wv, Wo):
    cos2 = np.asarray(cos, np.float32).reshape(T, HALF)  # [T, 32]
    sin2 = np.asarray(sin, np.float32).reshape(T, HALF)
    atab = np.tile(cos2.T, (4, 1))  # [128, T]
    btab = np.tile(np.vstack([sin2.T, -sin2.T]), (2, 1))  # [128, T]
    idx = np.arange(128)
    pswap = np.zeros((128, 128), np.float32)
    pswap[idx ^ 32, idx] = 1.0
    k_i = np.arange(128)[:, None]
    q_i = np.arange(128)[None, :]
    trib = np.where(k_i > q_i, np.float32(NEG), np.float32(0.0))
    import ml_dtypes

    trib = trib.astype(ml_dtypes.bfloat16)
    identb = np.eye(128, dtype=ml_dtypes.bfloat16)
    identr = np.eye(64, dtype=np.float32)
    ones65 = np.ones((65, 64), np.float32)
    ones16 = np.ones((128, 16), np.float32)
    zeros64 = np.zeros((64, T), np.float32)

    in_maps = []
    for core in range(8):
        b, g = core // 4, core % 4
        xt = np.ascontiguousarray(np.asarray(x[b], np.float32).T)  # [C, T]
        wproj = np.ascontiguousarray(
            np.concatenate(
                [
                    Wq[:, g * FQ : (g + 1) * FQ],
                    Wk[:, g * D : (g + 1) * D],
                    Wv[:, g * D : (g + 1) * D],
                ],
                axis=1,
            ).astype(np.float32)
        )
        wo = np.ascontiguousarray(Wo[g * FQ : (g + 1) * FQ, :].astype(np.float32))
        in_maps.append(
            {
                "xt": xt,
                "wproj": wproj,
                "wo": wo,
                "atab": atab,
                "btab": btab,
                "pswap": pswap,
                "trib": trib,
                "identb": identb,
                "identr": identr,
                "ones65": ones65,
                "ones16": ones16,
                "zeros64": zeros64,
            }
        )
    return in_maps


def kernel(x, cos, sin, Wq, Wk, Wv, Wo, _want_trace=False, _trace_kwargs=None):
    nc = _get_nc()
    in_maps = _host_prep(x, cos, sin, Wq, Wk, Wv, Wo)
    kw = {}
    if _want_trace:
        kw = dict(trace=True, **(_trace_kwargs or {}))
    res = run_bass_kernel_spmd(nc, in_maps, list(range(8)), **kw)
    y = np.zeros((B, T, C), np.float32)
    for core in range(8):
        b = core // 4
        y[b] += res.results[core]["y"]
    if _want_trace:
        kernel.last_result = res
    return y
